# revision 20
# baseline (speedup 1.0000x reference)
"""AttentionConv Trainium2 kernel (8 NeuronCores, data-parallel over batch).

Reference math (per batch b, channel o, position (h,w), 7x7 window d=(di,dj)):
    q = wq @ x, k = wk @ x, v = wv @ x   (1x1 convs, channel matmuls)
    logits_d = q * k_d + rel             (k_d = zero-padded k shifted by d)
    out = sum_d softmax_d(logits) * v_d
`rel` does not depend on d, so it cancels in the softmax. With zero padding,
out-of-bounds taps contribute exp(0)=1 to the denominator and 0 to the
numerator, which the zero-padded k/v tiles reproduce exactly.

Per-core schedule (engine-balanced against the DVE wall):
  PE : prewarm dummies (HAM un-throttle), 3 channel-projection matmuls,
       then per-offset identity-matmul accumulation of den += e_d and
       num += e_d*v_d into PSUM (4+4 512-col MMs per offset; redundant
       LDWEIGHTS are deduped post-legalization — see _dedup_ldweights).
  DVE: per offset-batch, t = q*k_d and u = e_d*v_d (bf16 tensor_tensor,
       2x mode); offsets with the same di and same dj parity are batched
       into one instruction via a stride-2 AP on the dj axis and a
       broadcast AP on q. Tail 1/den via RECIPROCAL_APPROX_FAST custom op
       (no ACT table loads). Plus the g0/q halves of PSUM evacuations.
  ACT: exp over whole offset-batches (bf16) + g1 PSUM evacuations.
  DMA: x loaded bf16-cast in the SWDGE datapath (gpsimd queue); weights
       on the scalar queue; shifted twin slabs built SBUF->SBUF on the
       sync queue hidden under the odd-dj half of the loop; output stored
       per-chunk on alternating queues.
The first PRO odd-dj batches are emitted before the v projection so the
loop ramps while v is still being produced.
Partition layout: p = g*64 + o for H-halves g in {0,1}; free dim = (32,64).
Padded k/v slabs are [38 rows x 72 cols] per partition with 1-element-shifted
twins so even dj offsets stay 4-byte aligned (keeps DVE 2x packing).
Odd-dj batches run first so the DMA-built twins are off the critical path.
"""

import numpy as np
import ml_dtypes

import concourse.bass as bass
import concourse.tile as tile
from concourse import bacc, mybir
from concourse.bass_utils import run_bass_kernel_spmd

N_CORES = 8
B, C, H, W, O = 8, 64, 64, 64, 64
KS, PAD = 7, 3
HW = H * W                      # 4096
HG = H // 2                     # 32 rows per partition group
NHALF = HG * W                  # 2048 free elements per partition
RSLAB = HG + KS - 1             # 38 padded slab rows
LPAD = 4                        # left col pad (even so evac dests align)
CSLAB = W + LPAD + PAD + 1      # 72 cols (even row stride)
SLAB = RSLAB * CSLAB            # 2736

F32 = mybir.dt.float32
BF16 = mybir.dt.bfloat16
_NPBF16 = ml_dtypes.bfloat16

# tunables
CFG = {
    "mmcols": 512,    # moving-operand width (ISA caps MM free size at 512)
    "warm_mms": 14,    # dummy matmuls to lift the PE HAM throttle early
    "tbufs": 3, "ebufs": 3, "ubufs": 4,
}


def _dedup_ldweights(nc):
    """Drop PE weight reloads whose stationary matches the already-loaded
    one.  Legalization pairs every matmul with its own InstLdweights; the
    49-offset accumulation loop reuses one identity stationary, so ~390 of
    the ~420 loads are redundant (~100ns each on the PE queue).  PE weights
    persist in the array and same-engine order is program order, so a
    duplicate load with no sync_info of its own can simply be removed.
    move_matmul_waits_to_ldweights is disabled alongside (a matmul's waits
    must not migrate to a far-earlier surviving load)."""
    removed = 0
    for b in nc.main_func.blocks:
        cur = None
        keep = []
        for i in b.instructions:
            tn = type(i).__name__
            if tn == "InstLdweights":
                sig = (str(i.ins[0]), str(i.tile_position), str(i.tile_size),
                       str(i.perf_mode), str(i.is_transpose))
                si = getattr(i, "sync_info", None)
                clean = si is None or (not si.on_wait and not si.on_update)
                if sig == cur and clean:
                    removed += 1
                    continue
                cur = sig
            keep.append(i)
        if removed and len(keep) != len(b.instructions):
            b.instructions[:] = keep
    return removed


def build_program():
    nc = bacc.Bacc("TRN2", target_bir_lowering=False, debug=False,
                   num_devices=N_CORES)

    x_d = nc.dram_tensor("x", [C, HW], F32, kind="ExternalInput").ap()
    wqT_d = nc.dram_tensor("wqT", [C, O], BF16, kind="ExternalInput").ap()
    wkT_d = nc.dram_tensor("wkT", [C, O], BF16, kind="ExternalInput").ap()
    wvT_d = nc.dram_tensor("wvT", [C, O], BF16, kind="ExternalInput").ap()
    ident_d = nc.dram_tensor("ident", [128, 128], BF16, kind="ExternalInput").ap()
    out_d = nc.dram_tensor("out", [O, H, W], BF16, kind="ExternalOutput").ap()

    with tile.TileContext(nc) as tc:
        _build(tc, x_d, wqT_d, wkT_d, wvT_d, ident_d, out_d)

    n = _dedup_ldweights(nc)
    assert n > 300, f"ldweights dedup removed only {n}"
    nc.move_matmul_waits_to_ldweights = lambda: None
    nc.compile()
    return nc


def _build(tc, x_d, wqT_d, wkT_d, wvT_d, ident_d, out_d):
    nc = tc.nc
    from contextlib import ExitStack

    with ExitStack() as ctx:
        konst = ctx.enter_context(tc.tile_pool(name="konst", bufs=1))
        big = ctx.enter_context(tc.tile_pool(name="big", bufs=1))

        # --- small constants on the scalar HWDGE queue; x arrives already
        # cast to bf16 via the gpsimd (SWDGE) queue, which converts dtypes
        # in the DMA datapath — no engine cast passes at all. ---
        wkT_sb = konst.tile([C, O], BF16)
        nc.scalar.dma_start(wkT_sb[:], wkT_d[:])
        wqT_sb = konst.tile([C, O], BF16)
        nc.scalar.dma_start(wqT_sb[:], wqT_d[:])
        wvT_sb = konst.tile([C, O], BF16)
        nc.scalar.dma_start(wvT_sb[:], wvT_d[:])
        ident_sb = konst.tile([128, 128], BF16)
        nc.scalar.dma_start(ident_sb[:], ident_d[:])

        tpool = ctx.enter_context(tc.tile_pool(name="tpool", bufs=CFG["tbufs"]))
        epool = ctx.enter_context(tc.tile_pool(name="epool", bufs=CFG["ebufs"]))
        upool = ctx.enter_context(tc.tile_pool(name="upool", bufs=CFG["ubufs"]))

        head = ExitStack()
        head_pool = head.enter_context(tc.tile_pool(name="head", bufs=1))
        xb = head_pool.tile([C, HW], BF16)
        x_pieces = [(0, 8), (29, 8), (8, 21), (37, 27)]   # (row0, nrows)
        for r0, nr in x_pieces:
            sl = slice(r0 * W, (r0 + nr) * W)
            nc.gpsimd.dma_start(xb[:, sl], x_d[:, sl])

        # --- PE prewarm: dummy matmuls on a zeroed stationary so the HAM
        # clock gate lifts (1.2 -> 2.4 GHz) before the projections. ---
        warm_sb = konst.tile([128, 512], BF16)
        nc.vector.memset(warm_sb[:], 0.0)
        warm_ctx = ExitStack()
        warm_psum = warm_ctx.enter_context(
            tc.tile_pool(name="warmps", bufs=1, space="PSUM"))
        wps = warm_psum.tile([128, 512], F32)
        for i in range(CFG["warm_mms"]):
            nc.tensor.matmul(wps[:], warm_sb[:, 0:128], warm_sb[:],
                             start=True, stop=True)

        # --- padded k/v slabs (zeroed borders) + q ---
        q_sb = big.tile([128, HG, W], BF16)
        kp = big.tile([128, SLAB], BF16)
        vp = big.tile([128, SLAB], BF16)
        kps = big.tile([128, SLAB], BF16)
        vps = big.tile([128, SLAB], BF16)
        kp3 = kp.rearrange("p (r c) -> p r c", r=RSLAB)
        vp3 = vp.rearrange("p (r c) -> p r c", r=RSLAB)
        # only the borders need zeroing (interior is overwritten by evacs)
        for t3 in (kp3, vp3):
            nc.gpsimd.memset(t3[:, 0:PAD, :], 0.0)
            nc.gpsimd.memset(t3[:, RSLAB - PAD:RSLAB, :], 0.0)
            nc.gpsimd.memset(t3[:, PAD:RSLAB - PAD, 0:LPAD], 0.0)
            nc.gpsimd.memset(t3[:, PAD:RSLAB - PAD, LPAD + W:CSLAB], 0.0)

        kps3 = kps.rearrange("p (r c) -> p r c", r=RSLAB)
        vps3 = vps.rearrange("p (r c) -> p r c", r=RSLAB)

        proj_ctx = ExitStack()
        psum = proj_ctx.enter_context(
            tc.tile_pool(name="psum", bufs=3, space="PSUM"))

        # k/v projections into padded slabs (512-col chunks, ISA cap).
        # group 0 slab rows r hold image rows r-3 (valid r in [3,38));
        # group 1 slab rows r hold image rows r+29 (valid r in [0,35)).
        chunk_rows = [(0, 8), (8, 8), (16, 8), (24, 8), (32, 3)]

        def project_kv(wT_sb, dst3, name, evac_engines):
            # evac_engines: per (chunk, g) -> engine for the PSUM->SBUF copy
            for ci, (r0, nr) in enumerate(chunk_rows):
                n = nr * W
                ps = psum.tile([128, 512], F32, tag="proj",
                               name=f"{name}_ps{ci}")
                nc.tensor.matmul(ps[0:64, :n], wT_sb[:],
                                 xb[:, r0 * W:(r0 + nr) * W],
                                 start=True, stop=True)
                nc.tensor.matmul(ps[64:128, :n], wT_sb[:],
                                 xb[:, (29 + r0) * W:(29 + r0 + nr) * W],
                                 start=True, stop=True)
                src = ps[:, :n].rearrange("p (a b) -> p a b", a=nr)
                ev0 = evac_engines(ci, 0)
                ev1 = evac_engines(ci, 1)
                ev0(dst3[0:64, 3 + r0:3 + r0 + nr, LPAD:LPAD + W], src[0:64])
                ev1(dst3[64:128, r0:r0 + nr, LPAD:LPAD + W], src[64:128])

        # k first: the loop's first tensor_tensor needs kp.
        # Big evacs split DVE(g0)/ACT(g1) so neither engine serializes.
        def kv_evac(ci, g):
            return nc.vector.tensor_copy if g == 0 else nc.scalar.copy
        project_kv(wkT_sb, kp3, "k", kv_evac)

        # q projection: group g covers output rows [g*32, g*32+32).
        # All q evacs ride DVE — it is idle in this window and q gates t0.
        for cchunk in range(4):
            ps = psum.tile([128, 512], F32, tag="proj", name=f"q_ps{cchunk}")
            for g in (0, 1):
                rhs = xb[:, g * NHALF + cchunk * 512:
                         g * NHALF + (cchunk + 1) * 512]
                nc.tensor.matmul(ps[g * 64:(g + 1) * 64, :], wqT_sb[:], rhs,
                                 start=True, stop=True)
            dst = q_sb[:, cchunk * 8:(cchunk + 1) * 8, :]
            src = ps[:].rearrange("p (a b) -> p a b", a=8)
            if cchunk < 2:
                nc.vector.tensor_copy(dst, src)
            else:
                nc.scalar.copy(dst, src)

        # --- loop structures (the first odd-dj batches are emitted between
        # q and the v projection so DVE/ACT ramp while v is produced; the
        # den/num PSUM pool can only open once the projection PSUM closes,
        # so it is late-bound via `accref`) ---
        accref = {}

        # view col base is dj + LPAD - PAD = dj+1: odd dj is 4B-aligned
        # (direct), even dj reads the 1-shifted twin. A batch covers djs with
        # one stride-2 AP; narrow batches at the ends shorten fill/drain.
        batches = [(0, [1], False), (0, [3], False), (0, [5], False),
                   (1, [1, 3, 5], False), (2, [1, 3, 5], False)]
        for di in range(KS):
            even = [0, 2, 4, 6]
            if di == 0:
                batches += [(di, even[:2], True), (di, even[2:], True)]
            elif di == KS - 1:
                batches += [(di, even[:2], True), (di, [4], True),
                            (di, [6], True)]
            else:
                batches.append((di, even, True))
        batches += [(3, [1, 3, 5], False), (4, [1, 3, 5], False),
                    (5, [1, 3, 5], False), (6, [1, 3], False),
                    (6, [5], False)]
        n_off = sum(len(b[1]) for b in batches)
        assert n_off == KS * KS

        # A batch view [128, w, HG, W] covers w djs spaced 2 apart: the dj
        # axis is a stride-2 column walk, expressed as a hand-built AP on the
        # slab tensor (offsets in elements relative to the slab's own AP).
        from concourse.ap import AP as _AP

        def shifted_batch(base3, di, c0, w):
            full = base3.rearrange("p r c -> p (r c)")
            ppair = list(list(p) for p in full.ap)[0]
            return _AP(full.tensor, full.offset + di * CSLAB + c0,
                       [ppair, [2, w], [CSLAB, HG], [1, W]])

        mmc = CFG["mmcols"]
        n_mm = NHALF // mmc
        t_tiles, e_tiles = {}, {}
        off_base = {}
        _nxt = [0]

        def emit_te(bi):
            di, djs, use_twin = batches[bi]
            w = len(djs)
            c0 = djs[0] if use_twin else djs[0] + LPAD - PAD
            kv = shifted_batch(kps3 if use_twin else kp3, di, c0, w)
            t_t = tpool.tile([128, w, HG, W], BF16, tag="t", name=f"t{bi}")
            q_b = q_sb[:].unsqueeze(1).broadcast_to([128, w, HG, W])
            nc.vector.tensor_mul(t_t[:], q_b, kv)
            e_t = epool.tile([128, w, HG, W], BF16, tag="e", name=f"e{bi}")
            nc.scalar.activation(e_t[:], t_t[:],
                                 mybir.ActivationFunctionType.Exp)
            e_tiles[bi] = e_t
            off_base[bi] = _nxt[0]
            _nxt[0] += w

        def emit_u_mm(bi):
            den_ps, num_ps = accref["den"], accref["num"]
            di, djs, use_twin = batches[bi]
            w = len(djs)
            c0 = djs[0] if use_twin else djs[0] + LPAD - PAD
            vv = shifted_batch(vps3 if use_twin else vp3, di, c0, w)
            e_t = e_tiles.pop(bi)
            u_t = upool.tile([128, w, HG, W], BF16, tag="u", name=f"u{bi}")
            nc.vector.tensor_mul(u_t[:], e_t[:], vv)
            for j in range(w):
                first = off_base[bi] + j == 0
                last = off_base[bi] + j == n_off - 1
                ej = e_t[:, j].rearrange("p r c -> p (r c)")
                uj = u_t[:, j].rearrange("p r c -> p (r c)")
                for cc in range(n_mm):
                    sl = slice(cc * mmc, (cc + 1) * mmc)
                    nc.tensor.matmul(den_ps[:, sl], ident_sb[:], ej[:, sl],
                                     start=first, stop=last,
                                     skip_group_check=True)
                    nc.tensor.matmul(num_ps[:, sl], ident_sb[:], uj[:, sl],
                                     start=first, stop=last,
                                     skip_group_check=True)

        PRO = 3     # prologue batches emitted before the v projection
        for bi in range(PRO):
            emit_te(bi)

        def kv_evac_v(ci, g):
            if g == 0 and ci < 2:
                return nc.vector.tensor_copy
            return nc.scalar.copy
        project_kv(wvT_sb, vp3, "v", kv_evac_v)

        # shifted twins built by the DMA engines (SBUF->SBUF, sync queue),
        # hidden under the first (odd-dj) half of the loop.
        nc.sync.dma_start(kps[:, 0:SLAB - 1], kp[:, 1:SLAB])
        nc.sync.dma_start(vps[:, 0:SLAB - 1], vp[:, 1:SLAB])

        proj_ctx.close()
        warm_ctx.close()
        head.close()

        acc = ctx.enter_context(tc.tile_pool(name="acc", bufs=1, space="PSUM"))
        accref["den"] = acc.tile([128, NHALF], F32, name="den_ps")
        accref["num"] = acc.tile([128, NHALF], F32, name="num_ps")

        for bi in range(PRO):
            emit_u_mm(bi)
        for bi in range(PRO, len(batches)):
            emit_te(bi)
            emit_u_mm(bi)

        # --- divide and store ---
        # 1/den on DVE via the fast custom reciprocal (no ACT tables, ACT
        # stays free); per-chunk so recip/mul/DMA overlap the loop drain.
        tail_pool = ctx.enter_context(tc.tile_pool(name="tail", bufs=1))
        den_r = tail_pool.tile([128, NHALF], F32)
        out_sb = tail_pool.tile([128, NHALF], BF16)
        out3 = out_sb.rearrange("p (a b) -> p a b", a=HG)
        for cc in range(4):
            sl = slice(cc * 512, (cc + 1) * 512)
            nc.vector.reciprocal_approx_fast(den_r[:, sl],
                                             accref["den"][:, sl])
            nc.vector.tensor_mul(out_sb[:, sl], accref["num"][:, sl],
                                 den_r[:, sl])
            rsl = slice(cc * 8, (cc + 1) * 8)
            eng = nc.sync if cc % 2 == 0 else nc.scalar
            eng.dma_start(out_d[:, rsl, :], out3[0:64, rsl, :])
            eng.dma_start(out_d[:, HG + cc * 8:HG + (cc + 1) * 8, :],
                          out3[64:128, rsl, :])


_NC_CACHE = None


def _get_nc():
    global _NC_CACHE
    if _NC_CACHE is None:
        _NC_CACHE = build_program()
    return _NC_CACHE


def prepare_in_maps(x, wq, wk, wv):
    x = np.ascontiguousarray(np.asarray(x, dtype=np.float32))
    wqT = np.ascontiguousarray(np.asarray(wq, np.float32).T.astype(_NPBF16))
    wkT = np.ascontiguousarray(np.asarray(wk, np.float32).T.astype(_NPBF16))
    wvT = np.ascontiguousarray(np.asarray(wv, np.float32).T.astype(_NPBF16))
    ident = np.eye(128, dtype=_NPBF16)
    return [
        {"x": x[i].reshape(C, HW), "wqT": wqT, "wkT": wkT, "wvT": wvT,
         "ident": ident}
        for i in range(x.shape[0])
    ]


def run(in_maps, **kw):
    nc = _get_nc()
    return run_bass_kernel_spmd(nc, in_maps, list(range(N_CORES)), **kw)


def kernel(x, wq, wk, wv, rel_w=None, rel_h=None, kernel_size=7, padding=3,
           **_ignored):
    # rel_w/rel_h are constant along the softmax axis, so they cancel.
    assert int(kernel_size) == KS and int(padding) == PAD
    res = run(prepare_in_maps(x, wq, wk, wv))
    out = np.stack([res.results[i]["out"] for i in range(N_CORES)], axis=0)
    return out.astype(np.float32)


if __name__ == "__main__":
    rng = np.random.default_rng(0)
    x = rng.standard_normal((B, C, H, W), dtype=np.float32)
    wq = (rng.standard_normal((O, C)) * 0.1).astype(np.float32)
    wk = (rng.standard_normal((O, C)) * 0.1).astype(np.float32)
    wv = (rng.standard_normal((O, C)) * 0.1).astype(np.float32)
    out = kernel(x, wq, wk, wv)
    print("out", out.shape, out.dtype, float(np.abs(out).max()))


# revision 21
# speedup vs baseline: 1.2045x; 1.2045x over previous
"""AttentionConv Trainium2 kernel (8 NeuronCores, data-parallel over batch).

Reference math (per batch b, channel o, position (h,w), 7x7 window d=(di,dj)):
    q = wq @ x, k = wk @ x, v = wv @ x   (1x1 convs, channel matmuls)
    logits_d = q * k_d + rel             (k_d = zero-padded k shifted by d)
    out = sum_d softmax_d(logits) * v_d
`rel` does not depend on d, so it cancels in the softmax. With zero padding,
out-of-bounds taps contribute exp(0)=1 to the denominator and 0 to the
numerator, which the zero-padded k/v tiles reproduce exactly.

Per-core schedule (engine-balanced against the DVE wall):
  PE : prewarm dummies (HAM un-throttle), 3 channel-projection matmuls,
       then per-offset identity-matmul accumulation of den += e_d and
       num += e_d*v_d into PSUM (4+4 512-col MMs per offset; redundant
       LDWEIGHTS are deduped post-legalization — see _dedup_ldweights).
  DVE: per offset-batch, t = q*k_d and u = e_d*v_d (bf16 tensor_tensor,
       2x mode); offsets with the same di and same dj parity are batched
       into one instruction via a stride-2 AP on the dj axis and a
       broadcast AP on q. Tail 1/den via RECIPROCAL_APPROX_FAST custom op
       (no ACT table loads). Plus the g0/q halves of PSUM evacuations.
  ACT: exp over whole offset-batches (bf16) + g1 PSUM evacuations.
  DMA: x loaded bf16-cast in the SWDGE datapath (gpsimd queue); weights
       on the scalar queue; shifted twin slabs built SBUF->SBUF on the
       sync queue hidden under the odd-dj half of the loop; output stored
       per-chunk on alternating queues.
The first PRO odd-dj batches are emitted before the v projection so the
loop ramps while v is still being produced.
Partition layout: p = g*64 + o for H-halves g in {0,1}; free dim = (32,64).
Padded k/v slabs are [38 rows x 72 cols] per partition with 1-element-shifted
twins so even dj offsets stay 4-byte aligned (keeps DVE 2x packing).
Odd-dj batches run first so the DMA-built twins are off the critical path.
"""

import numpy as np
import ml_dtypes

import concourse.bass as bass
import concourse.tile as tile
from concourse import bacc, mybir
from concourse.bass_utils import run_bass_kernel_spmd

N_CORES = 8
B, C, H, W, O = 8, 64, 64, 64, 64
KS, PAD = 7, 3
HW = H * W                      # 4096
HG = H // 2                     # 32 rows per partition group
NHALF = HG * W                  # 2048 free elements per partition
RSLAB = HG + KS - 1             # 38 padded slab rows
LPAD = 4                        # left col pad (even so evac dests align)
CSLAB = W + LPAD + PAD + 1      # 72 cols (even row stride)
SLAB = RSLAB * CSLAB            # 2736

F32 = mybir.dt.float32
BF16 = mybir.dt.bfloat16
_NPBF16 = ml_dtypes.bfloat16

# tunables
CFG = {
    "mmcols": 512,    # moving-operand width (ISA caps MM free size at 512)
    "warm_mms": 14,    # dummy matmuls to lift the PE HAM throttle early
    "tbufs": 3, "ebufs": 3, "ubufs": 4,
}


def _dedup_ldweights(nc):
    """Drop PE weight reloads whose stationary matches the already-loaded
    one.  Legalization pairs every matmul with its own InstLdweights; the
    49-offset accumulation loop reuses one identity stationary, so ~390 of
    the ~420 loads are redundant (~100ns each on the PE queue).  PE weights
    persist in the array and same-engine order is program order, so a
    duplicate load with no sync_info of its own can simply be removed.
    move_matmul_waits_to_ldweights is disabled alongside (a matmul's waits
    must not migrate to a far-earlier surviving load)."""
    removed = 0
    for b in nc.main_func.blocks:
        cur = None
        keep = []
        for i in b.instructions:
            tn = type(i).__name__
            if tn == "InstLdweights":
                sig = (str(i.ins[0]), str(i.tile_position), str(i.tile_size),
                       str(i.perf_mode), str(i.is_transpose))
                si = getattr(i, "sync_info", None)
                clean = si is None or (not si.on_wait and not si.on_update)
                if sig == cur and clean:
                    removed += 1
                    continue
                cur = sig
            keep.append(i)
        if removed and len(keep) != len(b.instructions):
            b.instructions[:] = keep
    return removed


def build_program():
    nc = bacc.Bacc("TRN2", target_bir_lowering=False, debug=False,
                   num_devices=N_CORES)

    x_d = nc.dram_tensor("x", [C, HW], F32, kind="ExternalInput").ap()
    wqT_d = nc.dram_tensor("wqT", [C, O], BF16, kind="ExternalInput").ap()
    wkT_d = nc.dram_tensor("wkT", [C, O], BF16, kind="ExternalInput").ap()
    wvT_d = nc.dram_tensor("wvT", [C, O], BF16, kind="ExternalInput").ap()
    ident_d = nc.dram_tensor("ident", [128, 128], BF16, kind="ExternalInput").ap()
    out_d = nc.dram_tensor("out", [O, H, W], BF16, kind="ExternalOutput").ap()

    with tile.TileContext(nc) as tc:
        _build(tc, x_d, wqT_d, wkT_d, wvT_d, ident_d, out_d)

    n = _dedup_ldweights(nc)
    assert n > 300, f"ldweights dedup removed only {n}"
    nc.move_matmul_waits_to_ldweights = lambda: None
    nc.compile()
    return nc


def _build(tc, x_d, wqT_d, wkT_d, wvT_d, ident_d, out_d):
    nc = tc.nc
    from contextlib import ExitStack

    with ExitStack() as ctx:
        konst = ctx.enter_context(tc.tile_pool(name="konst", bufs=1))
        big = ctx.enter_context(tc.tile_pool(name="big", bufs=1))

        # --- small constants on the scalar HWDGE queue; x arrives already
        # cast to bf16 via the gpsimd (SWDGE) queue, which converts dtypes
        # in the DMA datapath — no engine cast passes at all. ---
        wkT_sb = konst.tile([C, O], BF16)
        nc.scalar.dma_start(wkT_sb[:], wkT_d[:])
        wqT_sb = konst.tile([C, O], BF16)
        nc.scalar.dma_start(wqT_sb[:], wqT_d[:])
        wvT_sb = konst.tile([C, O], BF16)
        nc.scalar.dma_start(wvT_sb[:], wvT_d[:])
        ident_sb = konst.tile([128, 128], BF16)
        nc.scalar.dma_start(ident_sb[:], ident_d[:])

        tpool = ctx.enter_context(tc.tile_pool(name="tpool", bufs=CFG["tbufs"]))
        epool = ctx.enter_context(tc.tile_pool(name="epool", bufs=CFG["ebufs"]))
        upool = ctx.enter_context(tc.tile_pool(name="upool", bufs=CFG["ubufs"]))

        head = ExitStack()
        head_pool = head.enter_context(tc.tile_pool(name="head", bufs=1))
        xb = head_pool.tile([C, HW], BF16)
        for qtr in range(4):
            sl = slice(qtr * (HW // 4), (qtr + 1) * (HW // 4))
            nc.gpsimd.dma_start(xb[:, sl], x_d[:, sl])

        # --- PE prewarm: dummy matmuls on a zeroed stationary so the HAM
        # clock gate lifts (1.2 -> 2.4 GHz) before the projections. ---
        warm_sb = konst.tile([128, 512], BF16)
        nc.vector.memset(warm_sb[:], 0.0)
        warm_ctx = ExitStack()
        warm_psum = warm_ctx.enter_context(
            tc.tile_pool(name="warmps", bufs=1, space="PSUM"))
        wps = warm_psum.tile([128, 512], F32)
        for i in range(CFG["warm_mms"]):
            nc.tensor.matmul(wps[:], warm_sb[:, 0:128], warm_sb[:],
                             start=True, stop=True)

        # --- padded k/v slabs (zeroed borders) + q ---
        q_sb = big.tile([128, HG, W], BF16)
        kp = big.tile([128, SLAB], BF16)
        vp = big.tile([128, SLAB], BF16)
        kps = big.tile([128, SLAB], BF16)
        vps = big.tile([128, SLAB], BF16)
        kp3 = kp.rearrange("p (r c) -> p r c", r=RSLAB)
        vp3 = vp.rearrange("p (r c) -> p r c", r=RSLAB)
        # only the borders need zeroing (interior is overwritten by evacs)
        for t3 in (kp3, vp3):
            nc.gpsimd.memset(t3[:, 0:PAD, :], 0.0)
            nc.gpsimd.memset(t3[:, RSLAB - PAD:RSLAB, :], 0.0)
            nc.gpsimd.memset(t3[:, PAD:RSLAB - PAD, 0:LPAD], 0.0)
            nc.gpsimd.memset(t3[:, PAD:RSLAB - PAD, LPAD + W:CSLAB], 0.0)

        kps3 = kps.rearrange("p (r c) -> p r c", r=RSLAB)
        vps3 = vps.rearrange("p (r c) -> p r c", r=RSLAB)

        proj_ctx = ExitStack()
        psum = proj_ctx.enter_context(
            tc.tile_pool(name="psum", bufs=3, space="PSUM"))

        # k/v projections into padded slabs (512-col chunks, ISA cap).
        # group 0 slab rows r hold image rows r-3 (valid r in [3,38));
        # group 1 slab rows r hold image rows r+29 (valid r in [0,35)).
        chunk_rows = [(0, 8), (8, 8), (16, 8), (24, 8), (32, 3)]

        def project_kv(wT_sb, dst3, name, evac_engines):
            # evac_engines: per (chunk, g) -> engine for the PSUM->SBUF copy
            for ci, (r0, nr) in enumerate(chunk_rows):
                n = nr * W
                ps = psum.tile([128, 512], F32, tag="proj",
                               name=f"{name}_ps{ci}")
                nc.tensor.matmul(ps[0:64, :n], wT_sb[:],
                                 xb[:, r0 * W:(r0 + nr) * W],
                                 start=True, stop=True)
                nc.tensor.matmul(ps[64:128, :n], wT_sb[:],
                                 xb[:, (29 + r0) * W:(29 + r0 + nr) * W],
                                 start=True, stop=True)
                src = ps[:, :n].rearrange("p (a b) -> p a b", a=nr)
                ev0 = evac_engines(ci, 0)
                ev1 = evac_engines(ci, 1)
                ev0(dst3[0:64, 3 + r0:3 + r0 + nr, LPAD:LPAD + W], src[0:64])
                ev1(dst3[64:128, r0:r0 + nr, LPAD:LPAD + W], src[64:128])

        # k first: the loop's first tensor_tensor needs kp.
        # Big evacs split DVE(g0)/ACT(g1) so neither engine serializes.
        def kv_evac(ci, g):
            return nc.vector.tensor_copy if g == 0 else nc.scalar.copy
        project_kv(wkT_sb, kp3, "k", kv_evac)

        # q projection: group g covers output rows [g*32, g*32+32).
        # All q evacs ride DVE — it is idle in this window and q gates t0.
        for cchunk in range(4):
            ps = psum.tile([128, 512], F32, tag="proj", name=f"q_ps{cchunk}")
            for g in (0, 1):
                rhs = xb[:, g * NHALF + cchunk * 512:
                         g * NHALF + (cchunk + 1) * 512]
                nc.tensor.matmul(ps[g * 64:(g + 1) * 64, :], wqT_sb[:], rhs,
                                 start=True, stop=True)
            dst = q_sb[:, cchunk * 8:(cchunk + 1) * 8, :]
            src = ps[:].rearrange("p (a b) -> p a b", a=8)
            if cchunk < 2:
                nc.vector.tensor_copy(dst, src)
            else:
                nc.scalar.copy(dst, src)

        # --- loop structures (the first odd-dj batches are emitted between
        # q and the v projection so DVE/ACT ramp while v is produced; the
        # den/num PSUM pool can only open once the projection PSUM closes,
        # so it is late-bound via `accref`) ---
        accref = {}

        # view col base is dj + LPAD - PAD = dj+1: odd dj is 4B-aligned
        # (direct), even dj reads the 1-shifted twin. A batch covers djs with
        # one stride-2 AP; narrow batches at the ends shorten fill/drain.
        batches = []                          # (di, [dj...], use_twin)
        for di in range(KS):
            odd = [1, 3, 5]
            if di == 0:
                batches += [(di, [d], False) for d in odd]
            else:
                batches.append((di, odd, False))
        for di in range(KS):
            even = [0, 2, 4, 6]
            if di == 0:
                batches += [(di, even[:2], True), (di, even[2:], True)]
            elif di == KS - 1:
                batches += [(di, even[:2], True), (di, [4], True),
                            (di, [6], True)]
            else:
                batches.append((di, even, True))
        n_off = sum(len(b[1]) for b in batches)
        assert n_off == KS * KS

        # A batch view [128, w, HG, W] covers w djs spaced 2 apart: the dj
        # axis is a stride-2 column walk, expressed as a hand-built AP on the
        # slab tensor (offsets in elements relative to the slab's own AP).
        from concourse.ap import AP as _AP

        def shifted_batch(base3, di, c0, w):
            full = base3.rearrange("p r c -> p (r c)")
            ppair = list(list(p) for p in full.ap)[0]
            return _AP(full.tensor, full.offset + di * CSLAB + c0,
                       [ppair, [2, w], [CSLAB, HG], [1, W]])

        mmc = CFG["mmcols"]
        n_mm = NHALF // mmc
        t_tiles, e_tiles = {}, {}
        off_base = {}
        _nxt = [0]

        def emit_te(bi):
            di, djs, use_twin = batches[bi]
            w = len(djs)
            c0 = djs[0] if use_twin else djs[0] + LPAD - PAD
            kv = shifted_batch(kps3 if use_twin else kp3, di, c0, w)
            t_t = tpool.tile([128, w, HG, W], BF16, tag="t", name=f"t{bi}")
            q_b = q_sb[:].unsqueeze(1).broadcast_to([128, w, HG, W])
            nc.vector.tensor_mul(t_t[:], q_b, kv)
            e_t = epool.tile([128, w, HG, W], BF16, tag="e", name=f"e{bi}")
            nc.scalar.activation(e_t[:], t_t[:],
                                 mybir.ActivationFunctionType.Exp)
            e_tiles[bi] = e_t
            off_base[bi] = _nxt[0]
            _nxt[0] += w

        def emit_u_mm(bi):
            den_ps, num_ps = accref["den"], accref["num"]
            di, djs, use_twin = batches[bi]
            w = len(djs)
            c0 = djs[0] if use_twin else djs[0] + LPAD - PAD
            vv = shifted_batch(vps3 if use_twin else vp3, di, c0, w)
            e_t = e_tiles.pop(bi)
            u_t = upool.tile([128, w, HG, W], BF16, tag="u", name=f"u{bi}")
            nc.vector.tensor_mul(u_t[:], e_t[:], vv)
            for j in range(w):
                first = off_base[bi] + j == 0
                last = off_base[bi] + j == n_off - 1
                ej = e_t[:, j].rearrange("p r c -> p (r c)")
                uj = u_t[:, j].rearrange("p r c -> p (r c)")
                for cc in range(n_mm):
                    sl = slice(cc * mmc, (cc + 1) * mmc)
                    nc.tensor.matmul(den_ps[:, sl], ident_sb[:], ej[:, sl],
                                     start=first, stop=last,
                                     skip_group_check=True)
                    nc.tensor.matmul(num_ps[:, sl], ident_sb[:], uj[:, sl],
                                     start=first, stop=last,
                                     skip_group_check=True)

        PRO = 3     # prologue batches emitted before the v projection
        for bi in range(PRO):
            emit_te(bi)

        def kv_evac_v(ci, g):
            if g == 0 and ci < 2:
                return nc.vector.tensor_copy
            return nc.scalar.copy
        project_kv(wvT_sb, vp3, "v", kv_evac_v)

        # shifted twins built by the DMA engines (SBUF->SBUF, sync queue),
        # hidden under the first (odd-dj) half of the loop.
        nc.sync.dma_start(kps[:, 0:SLAB - 1], kp[:, 1:SLAB])
        nc.sync.dma_start(vps[:, 0:SLAB - 1], vp[:, 1:SLAB])

        proj_ctx.close()
        warm_ctx.close()
        head.close()

        acc = ctx.enter_context(tc.tile_pool(name="acc", bufs=1, space="PSUM"))
        accref["den"] = acc.tile([128, NHALF], F32, name="den_ps")
        accref["num"] = acc.tile([128, NHALF], F32, name="num_ps")

        for bi in range(PRO):
            emit_u_mm(bi)
        for bi in range(PRO, len(batches)):
            emit_te(bi)
            emit_u_mm(bi)

        # --- divide and store ---
        # 1/den on DVE via the fast custom reciprocal (no ACT tables, ACT
        # stays free); per-chunk so recip/mul/DMA overlap the loop drain.
        tail_pool = ctx.enter_context(tc.tile_pool(name="tail", bufs=1))
        den_r = tail_pool.tile([128, NHALF], F32)
        out_sb = tail_pool.tile([128, NHALF], BF16)
        out3 = out_sb.rearrange("p (a b) -> p a b", a=HG)
        for cc in range(4):
            sl = slice(cc * 512, (cc + 1) * 512)
            nc.vector.reciprocal_approx_fast(den_r[:, sl],
                                             accref["den"][:, sl])
            nc.vector.tensor_mul(out_sb[:, sl], accref["num"][:, sl],
                                 den_r[:, sl])
            rsl = slice(cc * 8, (cc + 1) * 8)
            eng = nc.sync if cc % 2 == 0 else nc.scalar
            eng.dma_start(out_d[:, rsl, :], out3[0:64, rsl, :])
            eng.dma_start(out_d[:, HG + cc * 8:HG + (cc + 1) * 8, :],
                          out3[64:128, rsl, :])


_NC_CACHE = None


def _get_nc():
    global _NC_CACHE
    if _NC_CACHE is None:
        _NC_CACHE = build_program()
    return _NC_CACHE


def prepare_in_maps(x, wq, wk, wv):
    x = np.ascontiguousarray(np.asarray(x, dtype=np.float32))
    wqT = np.ascontiguousarray(np.asarray(wq, np.float32).T.astype(_NPBF16))
    wkT = np.ascontiguousarray(np.asarray(wk, np.float32).T.astype(_NPBF16))
    wvT = np.ascontiguousarray(np.asarray(wv, np.float32).T.astype(_NPBF16))
    ident = np.eye(128, dtype=_NPBF16)
    return [
        {"x": x[i].reshape(C, HW), "wqT": wqT, "wkT": wkT, "wvT": wvT,
         "ident": ident}
        for i in range(x.shape[0])
    ]


def run(in_maps, **kw):
    nc = _get_nc()
    return run_bass_kernel_spmd(nc, in_maps, list(range(N_CORES)), **kw)


def kernel(x, wq, wk, wv, rel_w=None, rel_h=None, kernel_size=7, padding=3,
           **_ignored):
    # rel_w/rel_h are constant along the softmax axis, so they cancel.
    assert int(kernel_size) == KS and int(padding) == PAD
    res = run(prepare_in_maps(x, wq, wk, wv))
    out = np.stack([res.results[i]["out"] for i in range(N_CORES)], axis=0)
    return out.astype(np.float32)


if __name__ == "__main__":
    rng = np.random.default_rng(0)
    x = rng.standard_normal((B, C, H, W), dtype=np.float32)
    wq = (rng.standard_normal((O, C)) * 0.1).astype(np.float32)
    wk = (rng.standard_normal((O, C)) * 0.1).astype(np.float32)
    wv = (rng.standard_normal((O, C)) * 0.1).astype(np.float32)
    out = kernel(x, wq, wk, wv)
    print("out", out.shape, out.dtype, float(np.abs(out).max()))


# revision 22
# speedup vs baseline: 1.2062x; 1.0014x over previous
"""AttentionConv Trainium2 kernel (8 NeuronCores, data-parallel over batch).

Reference math (per batch b, channel o, position (h,w), 7x7 window d=(di,dj)):
    q = wq @ x, k = wk @ x, v = wv @ x   (1x1 convs, channel matmuls)
    logits_d = q * k_d + rel             (k_d = zero-padded k shifted by d)
    out = sum_d softmax_d(logits) * v_d
`rel` does not depend on d, so it cancels in the softmax. With zero padding,
out-of-bounds taps contribute exp(0)=1 to the denominator and 0 to the
numerator, which the zero-padded k/v tiles reproduce exactly.

Per-core schedule (engine-balanced against the DVE wall):
  PE : prewarm dummies (HAM un-throttle), 3 channel-projection matmuls,
       then per-offset identity-matmul accumulation of den += e_d and
       num += e_d*v_d into PSUM (4+4 512-col MMs per offset; redundant
       LDWEIGHTS are deduped post-legalization — see _dedup_ldweights).
  DVE: per offset-batch, t = q*k_d and u = e_d*v_d (bf16 tensor_tensor,
       2x mode); offsets with the same di and same dj parity are batched
       into one instruction via a stride-2 AP on the dj axis and a
       broadcast AP on q. Tail 1/den via RECIPROCAL_APPROX_FAST custom op
       (no ACT table loads). Plus the g0/q halves of PSUM evacuations.
  ACT: exp over whole offset-batches (bf16) + g1 PSUM evacuations.
  DMA: x loaded bf16-cast in the SWDGE datapath (gpsimd queue); weights
       on the scalar queue; shifted twin slabs built SBUF->SBUF on the
       sync queue hidden under the odd-dj half of the loop; output stored
       per-chunk on alternating queues.
The first PRO odd-dj batches are emitted before the v projection so the
loop ramps while v is still being produced.
Partition layout: p = g*64 + o for H-halves g in {0,1}; free dim = (32,64).
Padded k/v slabs are [38 rows x 72 cols] per partition with 1-element-shifted
twins so even dj offsets stay 4-byte aligned (keeps DVE 2x packing).
Odd-dj batches run first so the DMA-built twins are off the critical path.
"""

import numpy as np
import ml_dtypes

import concourse.bass as bass
import concourse.tile as tile
from concourse import bacc, mybir
from concourse.bass_utils import run_bass_kernel_spmd

N_CORES = 8
B, C, H, W, O = 8, 64, 64, 64, 64
KS, PAD = 7, 3
HW = H * W                      # 4096
HG = H // 2                     # 32 rows per partition group
NHALF = HG * W                  # 2048 free elements per partition
RSLAB = HG + KS - 1             # 38 padded slab rows
LPAD = 4                        # left col pad (even so evac dests align)
CSLAB = W + LPAD + PAD + 1      # 72 cols (even row stride)
SLAB = RSLAB * CSLAB            # 2736

F32 = mybir.dt.float32
BF16 = mybir.dt.bfloat16
_NPBF16 = ml_dtypes.bfloat16

# tunables
CFG = {
    "mmcols": 512,    # moving-operand width (ISA caps MM free size at 512)
    "warm_mms": 14,    # dummy matmuls to lift the PE HAM throttle early
    "tbufs": 3, "ebufs": 3, "ubufs": 4,
}


def _dedup_ldweights(nc):
    """Drop PE weight reloads whose stationary matches the already-loaded
    one.  Legalization pairs every matmul with its own InstLdweights; the
    49-offset accumulation loop reuses one identity stationary, so ~390 of
    the ~420 loads are redundant (~100ns each on the PE queue).  PE weights
    persist in the array and same-engine order is program order, so a
    duplicate load with no sync_info of its own can simply be removed.
    move_matmul_waits_to_ldweights is disabled alongside (a matmul's waits
    must not migrate to a far-earlier surviving load)."""
    removed = 0
    for b in nc.main_func.blocks:
        cur = None
        keep = []
        for i in b.instructions:
            tn = type(i).__name__
            if tn == "InstLdweights":
                sig = (str(i.ins[0]), str(i.tile_position), str(i.tile_size),
                       str(i.perf_mode), str(i.is_transpose))
                si = getattr(i, "sync_info", None)
                clean = si is None or (not si.on_wait and not si.on_update)
                if sig == cur and clean:
                    removed += 1
                    continue
                cur = sig
            keep.append(i)
        if removed and len(keep) != len(b.instructions):
            b.instructions[:] = keep
    return removed


def build_program():
    nc = bacc.Bacc("TRN2", target_bir_lowering=False, debug=False,
                   num_devices=N_CORES)

    x_d = nc.dram_tensor("x", [C, HW], F32, kind="ExternalInput").ap()
    wqT_d = nc.dram_tensor("wqT", [C, O], BF16, kind="ExternalInput").ap()
    wkT_d = nc.dram_tensor("wkT", [C, O], BF16, kind="ExternalInput").ap()
    wvT_d = nc.dram_tensor("wvT", [C, O], BF16, kind="ExternalInput").ap()
    ident_d = nc.dram_tensor("ident", [128, 128], BF16, kind="ExternalInput").ap()
    out_d = nc.dram_tensor("out", [O, H, W], BF16, kind="ExternalOutput").ap()

    with tile.TileContext(nc) as tc:
        _build(tc, x_d, wqT_d, wkT_d, wvT_d, ident_d, out_d)

    n = _dedup_ldweights(nc)
    assert n > 300, f"ldweights dedup removed only {n}"
    nc.move_matmul_waits_to_ldweights = lambda: None
    nc.compile()
    return nc


def _build(tc, x_d, wqT_d, wkT_d, wvT_d, ident_d, out_d):
    nc = tc.nc
    from contextlib import ExitStack

    with ExitStack() as ctx:
        konst = ctx.enter_context(tc.tile_pool(name="konst", bufs=1))
        big = ctx.enter_context(tc.tile_pool(name="big", bufs=1))

        # --- small constants on the scalar HWDGE queue; x arrives already
        # cast to bf16 via the gpsimd (SWDGE) queue, which converts dtypes
        # in the DMA datapath — no engine cast passes at all. ---
        wkT_sb = konst.tile([C, O], BF16)
        nc.scalar.dma_start(wkT_sb[:], wkT_d[:])
        wqT_sb = konst.tile([C, O], BF16)
        nc.scalar.dma_start(wqT_sb[:], wqT_d[:])
        wvT_sb = konst.tile([C, O], BF16)
        nc.scalar.dma_start(wvT_sb[:], wvT_d[:])
        ident_sb = konst.tile([128, 128], BF16)
        nc.scalar.dma_start(ident_sb[:], ident_d[:])

        tpool = ctx.enter_context(tc.tile_pool(name="tpool", bufs=CFG["tbufs"]))
        epool = ctx.enter_context(tc.tile_pool(name="epool", bufs=CFG["ebufs"]))
        upool = ctx.enter_context(tc.tile_pool(name="upool", bufs=CFG["ubufs"]))

        head = ExitStack()
        head_pool = head.enter_context(tc.tile_pool(name="head", bufs=1))
        xb = head_pool.tile([C, HW], BF16)
        for qtr in range(4):
            sl = slice(qtr * (HW // 4), (qtr + 1) * (HW // 4))
            nc.gpsimd.dma_start(xb[:, sl], x_d[:, sl])

        # --- PE prewarm: dummy matmuls on a zeroed stationary so the HAM
        # clock gate lifts (1.2 -> 2.4 GHz) before the projections. ---
        warm_sb = konst.tile([128, 512], BF16)
        nc.vector.memset(warm_sb[:], 0.0)
        warm_ctx = ExitStack()
        warm_psum = warm_ctx.enter_context(
            tc.tile_pool(name="warmps", bufs=1, space="PSUM"))
        wps = warm_psum.tile([128, 512], F32)
        for i in range(CFG["warm_mms"]):
            nc.tensor.matmul(wps[:], warm_sb[:, 0:128], warm_sb[:],
                             start=True, stop=True)

        # --- padded k/v slabs (zeroed borders) + q ---
        q_sb = big.tile([128, HG, W], BF16)
        kp = big.tile([128, SLAB], BF16)
        vp = big.tile([128, SLAB], BF16)
        kps = big.tile([128, SLAB], BF16)
        vps = big.tile([128, SLAB], BF16)
        kp3 = kp.rearrange("p (r c) -> p r c", r=RSLAB)
        vp3 = vp.rearrange("p (r c) -> p r c", r=RSLAB)
        # only the borders need zeroing (interior is overwritten by evacs)
        for t3 in (kp3, vp3):
            nc.gpsimd.memset(t3[:, 0:PAD, :], 0.0)
            nc.gpsimd.memset(t3[:, RSLAB - PAD:RSLAB, :], 0.0)
            nc.gpsimd.memset(t3[:, PAD:RSLAB - PAD, 0:LPAD], 0.0)
            nc.gpsimd.memset(t3[:, PAD:RSLAB - PAD, LPAD + W:CSLAB], 0.0)

        kps3 = kps.rearrange("p (r c) -> p r c", r=RSLAB)
        vps3 = vps.rearrange("p (r c) -> p r c", r=RSLAB)

        proj_ctx = ExitStack()
        psum = proj_ctx.enter_context(
            tc.tile_pool(name="psum", bufs=3, space="PSUM"))

        # k/v projections into padded slabs (512-col chunks, ISA cap).
        # group 0 slab rows r hold image rows r-3 (valid r in [3,38));
        # group 1 slab rows r hold image rows r+29 (valid r in [0,35)).
        chunk_rows = [(0, 8), (8, 8), (16, 8), (24, 8), (32, 3)]

        def project_kv(wT_sb, dst3, name, evac_engines):
            # evac_engines: per (chunk, g) -> engine for the PSUM->SBUF copy
            for ci, (r0, nr) in enumerate(chunk_rows):
                n = nr * W
                ps = psum.tile([128, 512], F32, tag="proj",
                               name=f"{name}_ps{ci}")
                nc.tensor.matmul(ps[0:64, :n], wT_sb[:],
                                 xb[:, r0 * W:(r0 + nr) * W],
                                 start=True, stop=True)
                nc.tensor.matmul(ps[64:128, :n], wT_sb[:],
                                 xb[:, (29 + r0) * W:(29 + r0 + nr) * W],
                                 start=True, stop=True)
                src = ps[:, :n].rearrange("p (a b) -> p a b", a=nr)
                ev0 = evac_engines(ci, 0)
                ev1 = evac_engines(ci, 1)
                ev0(dst3[0:64, 3 + r0:3 + r0 + nr, LPAD:LPAD + W], src[0:64])
                ev1(dst3[64:128, r0:r0 + nr, LPAD:LPAD + W], src[64:128])

        # k first: the loop's first tensor_tensor needs kp.
        # Big evacs split DVE(g0)/ACT(g1) so neither engine serializes.
        def kv_evac(ci, g):
            return nc.vector.tensor_copy if g == 0 else nc.scalar.copy
        project_kv(wkT_sb, kp3, "k", kv_evac)

        # q projection: group g covers output rows [g*32, g*32+32).
        # All q evacs ride DVE — it is idle in this window and q gates t0.
        for cchunk in range(4):
            ps = psum.tile([128, 512], F32, tag="proj", name=f"q_ps{cchunk}")
            for g in (0, 1):
                rhs = xb[:, g * NHALF + cchunk * 512:
                         g * NHALF + (cchunk + 1) * 512]
                nc.tensor.matmul(ps[g * 64:(g + 1) * 64, :], wqT_sb[:], rhs,
                                 start=True, stop=True)
            dst = q_sb[:, cchunk * 8:(cchunk + 1) * 8, :]
            src = ps[:].rearrange("p (a b) -> p a b", a=8)
            if cchunk < 2:
                nc.vector.tensor_copy(dst, src)
            else:
                nc.scalar.copy(dst, src)

        # --- loop structures (the first odd-dj batches are emitted between
        # q and the v projection so DVE/ACT ramp while v is produced; the
        # den/num PSUM pool can only open once the projection PSUM closes,
        # so it is late-bound via `accref`) ---
        accref = {}

        # view col base is dj + LPAD - PAD = dj+1: odd dj is 4B-aligned
        # (direct), even dj reads the 1-shifted twin. A batch covers djs with
        # one stride-2 AP; narrow batches at the ends shorten fill/drain.
        batches = []                          # (di, [dj...], use_twin)
        for di in range(KS):
            odd = [1, 3, 5]
            if di == 0:
                batches += [(di, [d], False) for d in odd]
            else:
                batches.append((di, odd, False))
        for di in range(KS):
            even = [0, 2, 4, 6]
            if di == 0:
                batches += [(di, even[:2], True), (di, even[2:], True)]
            elif di == KS - 1:
                batches += [(di, even[:2], True), (di, [4], True),
                            (di, [6], True)]
            else:
                batches.append((di, even, True))
        n_off = sum(len(b[1]) for b in batches)
        assert n_off == KS * KS

        # A batch view [128, w, HG, W] covers w djs spaced 2 apart: the dj
        # axis is a stride-2 column walk, expressed as a hand-built AP on the
        # slab tensor (offsets in elements relative to the slab's own AP).
        from concourse.ap import AP as _AP

        def shifted_batch(base3, di, c0, w):
            full = base3.rearrange("p r c -> p (r c)")
            ppair = list(list(p) for p in full.ap)[0]
            return _AP(full.tensor, full.offset + di * CSLAB + c0,
                       [ppair, [2, w], [CSLAB, HG], [1, W]])

        mmc = CFG["mmcols"]
        n_mm = NHALF // mmc
        t_tiles, e_tiles = {}, {}
        off_base = {}
        _nxt = [0]

        def emit_te(bi):
            di, djs, use_twin = batches[bi]
            w = len(djs)
            c0 = djs[0] if use_twin else djs[0] + LPAD - PAD
            kv = shifted_batch(kps3 if use_twin else kp3, di, c0, w)
            t_t = tpool.tile([128, w, HG, W], BF16, tag="t", name=f"t{bi}")
            q_b = q_sb[:].unsqueeze(1).broadcast_to([128, w, HG, W])
            nc.vector.tensor_mul(t_t[:], q_b, kv)
            e_t = epool.tile([128, w, HG, W], BF16, tag="e", name=f"e{bi}")
            nc.scalar.activation(e_t[:], t_t[:],
                                 mybir.ActivationFunctionType.Exp)
            e_tiles[bi] = e_t
            off_base[bi] = _nxt[0]
            _nxt[0] += w

        def emit_u_mm(bi):
            den_ps, num_ps = accref["den"], accref["num"]
            di, djs, use_twin = batches[bi]
            w = len(djs)
            c0 = djs[0] if use_twin else djs[0] + LPAD - PAD
            vv = shifted_batch(vps3 if use_twin else vp3, di, c0, w)
            e_t = e_tiles.pop(bi)
            u_t = upool.tile([128, w, HG, W], BF16, tag="u", name=f"u{bi}")
            nc.vector.tensor_mul(u_t[:], e_t[:], vv)
            for j in range(w):
                first = off_base[bi] + j == 0
                last = off_base[bi] + j == n_off - 1
                ej = e_t[:, j].rearrange("p r c -> p (r c)")
                uj = u_t[:, j].rearrange("p r c -> p (r c)")
                for cc in range(n_mm):
                    sl = slice(cc * mmc, (cc + 1) * mmc)
                    nc.tensor.matmul(den_ps[:, sl], ident_sb[:], ej[:, sl],
                                     start=first, stop=last,
                                     skip_group_check=True)
                    nc.tensor.matmul(num_ps[:, sl], ident_sb[:], uj[:, sl],
                                     start=first, stop=last,
                                     skip_group_check=True)

        PRO = 3     # prologue batches emitted before the v projection
        for bi in range(PRO):
            emit_te(bi)

        def kv_evac_v(ci, g):
            if g == 0 and ci < 2:
                return nc.vector.tensor_copy
            return nc.scalar.copy
        project_kv(wvT_sb, vp3, "v", kv_evac_v)

        # shifted twins built by the DMA engines (SBUF->SBUF, sync queue),
        # hidden under the first (odd-dj) half of the loop.
        nc.sync.dma_start(kps[:, 0:SLAB - 1], kp[:, 1:SLAB])
        nc.sync.dma_start(vps[:, 0:SLAB - 1], vp[:, 1:SLAB])

        proj_ctx.close()
        warm_ctx.close()
        head.close()

        acc = ctx.enter_context(tc.tile_pool(name="acc", bufs=1, space="PSUM"))
        accref["den"] = acc.tile([128, NHALF], F32, name="den_ps")
        accref["num"] = acc.tile([128, NHALF], F32, name="num_ps")

        # Software-pipeline by LAG batches: the DVE queue is FIFO, so an
        # emission order t_i,u_i would park u_i (whose e_i = exp(t_i) takes
        # ~7us on ACT) at the queue head and stall t_{i+1} behind it.  With
        # t_i,u_{i-LAG}, the exp a u waits on finished ~2 batches ago.
        LAG = 2
        emit_u_mm(0)
        for bi in range(PRO, len(batches)):
            emit_te(bi)
            emit_u_mm(bi - LAG)
        for bi in range(len(batches) - LAG, len(batches)):
            emit_u_mm(bi)

        # --- divide and store ---
        # 1/den on DVE via the fast custom reciprocal (no ACT tables, ACT
        # stays free); per-chunk so recip/mul/DMA overlap the loop drain.
        tail_pool = ctx.enter_context(tc.tile_pool(name="tail", bufs=1))
        den_r = tail_pool.tile([128, NHALF], F32)
        out_sb = tail_pool.tile([128, NHALF], BF16)
        out3 = out_sb.rearrange("p (a b) -> p a b", a=HG)
        for cc in range(4):
            sl = slice(cc * 512, (cc + 1) * 512)
            nc.vector.reciprocal_approx_fast(den_r[:, sl],
                                             accref["den"][:, sl])
            nc.vector.tensor_mul(out_sb[:, sl], accref["num"][:, sl],
                                 den_r[:, sl])
            rsl = slice(cc * 8, (cc + 1) * 8)
            eng = nc.sync if cc % 2 == 0 else nc.scalar
            eng.dma_start(out_d[:, rsl, :], out3[0:64, rsl, :])
            eng.dma_start(out_d[:, HG + cc * 8:HG + (cc + 1) * 8, :],
                          out3[64:128, rsl, :])


_NC_CACHE = None


def _get_nc():
    global _NC_CACHE
    if _NC_CACHE is None:
        _NC_CACHE = build_program()
    return _NC_CACHE


def prepare_in_maps(x, wq, wk, wv):
    x = np.ascontiguousarray(np.asarray(x, dtype=np.float32))
    wqT = np.ascontiguousarray(np.asarray(wq, np.float32).T.astype(_NPBF16))
    wkT = np.ascontiguousarray(np.asarray(wk, np.float32).T.astype(_NPBF16))
    wvT = np.ascontiguousarray(np.asarray(wv, np.float32).T.astype(_NPBF16))
    ident = np.eye(128, dtype=_NPBF16)
    return [
        {"x": x[i].reshape(C, HW), "wqT": wqT, "wkT": wkT, "wvT": wvT,
         "ident": ident}
        for i in range(x.shape[0])
    ]


def run(in_maps, **kw):
    nc = _get_nc()
    return run_bass_kernel_spmd(nc, in_maps, list(range(N_CORES)), **kw)


def kernel(x, wq, wk, wv, rel_w=None, rel_h=None, kernel_size=7, padding=3,
           **_ignored):
    # rel_w/rel_h are constant along the softmax axis, so they cancel.
    assert int(kernel_size) == KS and int(padding) == PAD
    res = run(prepare_in_maps(x, wq, wk, wv))
    out = np.stack([res.results[i]["out"] for i in range(N_CORES)], axis=0)
    return out.astype(np.float32)


if __name__ == "__main__":
    rng = np.random.default_rng(0)
    x = rng.standard_normal((B, C, H, W), dtype=np.float32)
    wq = (rng.standard_normal((O, C)) * 0.1).astype(np.float32)
    wk = (rng.standard_normal((O, C)) * 0.1).astype(np.float32)
    wv = (rng.standard_normal((O, C)) * 0.1).astype(np.float32)
    out = kernel(x, wq, wk, wv)
    print("out", out.shape, out.dtype, float(np.abs(out).max()))


# revision 23
# speedup vs baseline: 1.2160x; 1.0081x over previous
"""AttentionConv Trainium2 kernel (8 NeuronCores, data-parallel over batch).

Reference math (per batch b, channel o, position (h,w), 7x7 window d=(di,dj)):
    q = wq @ x, k = wk @ x, v = wv @ x   (1x1 convs, channel matmuls)
    logits_d = q * k_d + rel             (k_d = zero-padded k shifted by d)
    out = sum_d softmax_d(logits) * v_d
`rel` does not depend on d, so it cancels in the softmax. With zero padding,
out-of-bounds taps contribute exp(0)=1 to the denominator and 0 to the
numerator, which the zero-padded k/v tiles reproduce exactly.

Per-core schedule (engine-balanced against the DVE wall):
  PE : prewarm dummies (HAM un-throttle), 3 channel-projection matmuls,
       then per-offset identity-matmul accumulation of den += e_d and
       num += e_d*v_d into PSUM (4+4 512-col MMs per offset; redundant
       LDWEIGHTS are deduped post-legalization — see _dedup_ldweights).
  DVE: per offset-batch, t = q*k_d and u = e_d*v_d (bf16 tensor_tensor,
       2x mode); offsets with the same di and same dj parity are batched
       into one instruction via a stride-2 AP on the dj axis and a
       broadcast AP on q. Tail 1/den via RECIPROCAL_APPROX_FAST custom op
       (no ACT table loads). Plus the g0/q halves of PSUM evacuations.
  ACT: exp over whole offset-batches (bf16) + g1 PSUM evacuations.
  DMA: x loaded bf16-cast in the SWDGE datapath (gpsimd queue); weights
       on the scalar queue; shifted twin slabs built SBUF->SBUF on the
       sync queue hidden under the odd-dj half of the loop; output stored
       per-chunk on alternating queues.
The first PRO odd-dj batches are emitted before the v projection so the
loop ramps while v is still being produced.
Partition layout: p = g*64 + o for H-halves g in {0,1}; free dim = (32,64).
Padded k/v slabs are [38 rows x 72 cols] per partition with 1-element-shifted
twins so even dj offsets stay 4-byte aligned (keeps DVE 2x packing).
Odd-dj batches run first so the DMA-built twins are off the critical path.
"""

import numpy as np
import ml_dtypes

import concourse.bass as bass
import concourse.tile as tile
from concourse import bacc, mybir
from concourse.bass_utils import run_bass_kernel_spmd

N_CORES = 8
B, C, H, W, O = 8, 64, 64, 64, 64
KS, PAD = 7, 3
HW = H * W                      # 4096
HG = H // 2                     # 32 rows per partition group
NHALF = HG * W                  # 2048 free elements per partition
RSLAB = HG + KS - 1             # 38 padded slab rows
LPAD = 4                        # left col pad (even so evac dests align)
CSLAB = W + LPAD + PAD + 1      # 72 cols (even row stride)
SLAB = RSLAB * CSLAB            # 2736

F32 = mybir.dt.float32
BF16 = mybir.dt.bfloat16
_NPBF16 = ml_dtypes.bfloat16

# tunables
CFG = {
    "mmcols": 512,    # moving-operand width (ISA caps MM free size at 512)
    "warm_mms": 14,    # dummy matmuls to lift the PE HAM throttle early
    "tbufs": 3, "ebufs": 3, "ubufs": 4,
}


def _dedup_ldweights(nc):
    """Drop PE weight reloads whose stationary matches the already-loaded
    one.  Legalization pairs every matmul with its own InstLdweights; the
    49-offset accumulation loop reuses one identity stationary, so ~390 of
    the ~420 loads are redundant (~100ns each on the PE queue).  PE weights
    persist in the array and same-engine order is program order, so a
    duplicate load with no sync_info of its own can simply be removed.
    move_matmul_waits_to_ldweights is disabled alongside (a matmul's waits
    must not migrate to a far-earlier surviving load)."""
    removed = 0
    for b in nc.main_func.blocks:
        cur = None
        keep = []
        for i in b.instructions:
            tn = type(i).__name__
            if tn == "InstLdweights":
                sig = (str(i.ins[0]), str(i.tile_position), str(i.tile_size),
                       str(i.perf_mode), str(i.is_transpose))
                si = getattr(i, "sync_info", None)
                clean = si is None or (not si.on_wait and not si.on_update)
                if sig == cur and clean:
                    removed += 1
                    continue
                cur = sig
            keep.append(i)
        if removed and len(keep) != len(b.instructions):
            b.instructions[:] = keep
    return removed


def build_program():
    nc = bacc.Bacc("TRN2", target_bir_lowering=False, debug=False,
                   num_devices=N_CORES)

    x_d = nc.dram_tensor("x", [C, HW], F32, kind="ExternalInput").ap()
    wqT_d = nc.dram_tensor("wqT", [C, O], BF16, kind="ExternalInput").ap()
    wkT_d = nc.dram_tensor("wkT", [C, O], BF16, kind="ExternalInput").ap()
    wvT_d = nc.dram_tensor("wvT", [C, O], BF16, kind="ExternalInput").ap()
    ident_d = nc.dram_tensor("ident", [128, 128], BF16, kind="ExternalInput").ap()
    out_d = nc.dram_tensor("out", [O, H, W], BF16, kind="ExternalOutput").ap()

    with tile.TileContext(nc) as tc:
        _build(tc, x_d, wqT_d, wkT_d, wvT_d, ident_d, out_d)

    n = _dedup_ldweights(nc)
    assert n > 300, f"ldweights dedup removed only {n}"
    nc.move_matmul_waits_to_ldweights = lambda: None
    nc.compile()
    return nc


def _build(tc, x_d, wqT_d, wkT_d, wvT_d, ident_d, out_d):
    nc = tc.nc
    from contextlib import ExitStack

    with ExitStack() as ctx:
        konst = ctx.enter_context(tc.tile_pool(name="konst", bufs=1))
        big = ctx.enter_context(tc.tile_pool(name="big", bufs=1))

        # --- small constants on the scalar HWDGE queue; x arrives already
        # cast to bf16 via the gpsimd (SWDGE) queue, which converts dtypes
        # in the DMA datapath — no engine cast passes at all. ---
        wkT_sb = konst.tile([C, O], BF16)
        nc.scalar.dma_start(wkT_sb[:], wkT_d[:])
        wqT_sb = konst.tile([C, O], BF16)
        nc.scalar.dma_start(wqT_sb[:], wqT_d[:])
        wvT_sb = konst.tile([C, O], BF16)
        nc.scalar.dma_start(wvT_sb[:], wvT_d[:])
        ident_sb = konst.tile([128, 128], BF16)
        nc.scalar.dma_start(ident_sb[:], ident_d[:])

        tpool = ctx.enter_context(tc.tile_pool(name="tpool", bufs=CFG["tbufs"]))
        epool = ctx.enter_context(tc.tile_pool(name="epool", bufs=CFG["ebufs"]))
        upool = ctx.enter_context(tc.tile_pool(name="upool", bufs=CFG["ubufs"]))

        head = ExitStack()
        head_pool = head.enter_context(tc.tile_pool(name="head", bufs=1))
        xb = head_pool.tile([C, HW], BF16)
        for qtr in range(4):
            sl = slice(qtr * (HW // 4), (qtr + 1) * (HW // 4))
            nc.gpsimd.dma_start(xb[:, sl], x_d[:, sl])

        # --- PE prewarm: dummy matmuls on a zeroed stationary so the HAM
        # clock gate lifts (1.2 -> 2.4 GHz) before the projections. ---
        warm_sb = konst.tile([128, 512], BF16)
        nc.vector.memset(warm_sb[:], 0.0)
        warm_ctx = ExitStack()
        warm_psum = warm_ctx.enter_context(
            tc.tile_pool(name="warmps", bufs=1, space="PSUM"))
        wps = warm_psum.tile([128, 512], F32)
        for i in range(CFG["warm_mms"]):
            nc.tensor.matmul(wps[:], warm_sb[:, 0:128], warm_sb[:],
                             start=True, stop=True)

        # --- padded k/v slabs (zeroed borders) + q ---
        q_sb = big.tile([128, HG, W], BF16)
        kp = big.tile([128, SLAB], BF16)
        vp = big.tile([128, SLAB], BF16)
        kps = big.tile([128, SLAB], BF16)
        vps = big.tile([128, SLAB], BF16)
        kp3 = kp.rearrange("p (r c) -> p r c", r=RSLAB)
        vp3 = vp.rearrange("p (r c) -> p r c", r=RSLAB)
        # only the borders need zeroing (interior is overwritten by evacs)
        for t3 in (kp3, vp3):
            nc.gpsimd.memset(t3[:, 0:PAD, :], 0.0)
            nc.gpsimd.memset(t3[:, RSLAB - PAD:RSLAB, :], 0.0)
            nc.gpsimd.memset(t3[:, PAD:RSLAB - PAD, 0:LPAD], 0.0)
            nc.gpsimd.memset(t3[:, PAD:RSLAB - PAD, LPAD + W:CSLAB], 0.0)

        kps3 = kps.rearrange("p (r c) -> p r c", r=RSLAB)
        vps3 = vps.rearrange("p (r c) -> p r c", r=RSLAB)

        proj_ctx = ExitStack()
        psum = proj_ctx.enter_context(
            tc.tile_pool(name="psum", bufs=3, space="PSUM"))

        # k/v projections into padded slabs (512-col chunks, ISA cap).
        # group 0 slab rows r hold image rows r-3 (valid r in [3,38));
        # group 1 slab rows r hold image rows r+29 (valid r in [0,35)).
        chunk_rows = [(0, 8), (8, 8), (16, 8), (24, 8), (32, 3)]

        def project_kv(wT_sb, dst3, name, evac_engines):
            # evac_engines: per (chunk, g) -> engine for the PSUM->SBUF copy
            for ci, (r0, nr) in enumerate(chunk_rows):
                n = nr * W
                ps = psum.tile([128, 512], F32, tag="proj",
                               name=f"{name}_ps{ci}")
                nc.tensor.matmul(ps[0:64, :n], wT_sb[:],
                                 xb[:, r0 * W:(r0 + nr) * W],
                                 start=True, stop=True)
                nc.tensor.matmul(ps[64:128, :n], wT_sb[:],
                                 xb[:, (29 + r0) * W:(29 + r0 + nr) * W],
                                 start=True, stop=True)
                src = ps[:, :n].rearrange("p (a b) -> p a b", a=nr)
                ev0 = evac_engines(ci, 0)
                ev1 = evac_engines(ci, 1)
                ev0(dst3[0:64, 3 + r0:3 + r0 + nr, LPAD:LPAD + W], src[0:64])
                ev1(dst3[64:128, r0:r0 + nr, LPAD:LPAD + W], src[64:128])

        # k first: the loop's first tensor_tensor needs kp.
        # Big evacs split DVE(g0)/ACT(g1) so neither engine serializes.
        def kv_evac(ci, g):
            return nc.vector.tensor_copy if g == 0 else nc.scalar.copy
        project_kv(wkT_sb, kp3, "k", kv_evac)

        # q projection: group g covers output rows [g*32, g*32+32).
        # All q evacs ride DVE — it is idle in this window and q gates t0.
        for cchunk in range(4):
            ps = psum.tile([128, 512], F32, tag="proj", name=f"q_ps{cchunk}")
            for g in (0, 1):
                rhs = xb[:, g * NHALF + cchunk * 512:
                         g * NHALF + (cchunk + 1) * 512]
                nc.tensor.matmul(ps[g * 64:(g + 1) * 64, :], wqT_sb[:], rhs,
                                 start=True, stop=True)
            dst = q_sb[:, cchunk * 8:(cchunk + 1) * 8, :]
            src = ps[:].rearrange("p (a b) -> p a b", a=8)
            if cchunk < 2:
                nc.vector.tensor_copy(dst, src)
            else:
                nc.scalar.copy(dst, src)

        # --- loop structures (the first odd-dj batches are emitted between
        # q and the v projection so DVE/ACT ramp while v is produced; the
        # den/num PSUM pool can only open once the projection PSUM closes,
        # so it is late-bound via `accref`) ---
        accref = {}

        # view col base is dj + LPAD - PAD = dj+1: odd dj is 4B-aligned
        # (direct), even dj reads the 1-shifted twin. A batch covers djs with
        # one stride-2 AP; narrow batches at the ends shorten fill/drain.
        batches = []                          # (di, [dj...], use_twin)
        for di in range(KS):
            odd = [1, 3, 5]
            if di == 0:
                batches += [(di, [d], False) for d in odd]
            else:
                batches.append((di, odd, False))
        for di in range(KS):
            even = [0, 2, 4, 6]
            if di == 0:
                batches += [(di, even[:2], True), (di, even[2:], True)]
            elif di == KS - 1:
                batches += [(di, even[:2], True), (di, [4], True),
                            (di, [6], True)]
            else:
                batches.append((di, even, True))
        n_off = sum(len(b[1]) for b in batches)
        assert n_off == KS * KS

        # A batch view [128, w, HG, W] covers w djs spaced 2 apart: the dj
        # axis is a stride-2 column walk, expressed as a hand-built AP on the
        # slab tensor (offsets in elements relative to the slab's own AP).
        from concourse.ap import AP as _AP

        def shifted_batch(base3, di, c0, w):
            full = base3.rearrange("p r c -> p (r c)")
            ppair = list(list(p) for p in full.ap)[0]
            return _AP(full.tensor, full.offset + di * CSLAB + c0,
                       [ppair, [2, w], [CSLAB, HG], [1, W]])

        mmc = CFG["mmcols"]
        n_mm = NHALF // mmc
        t_tiles, e_tiles = {}, {}
        off_base = {}
        _nxt = [0]

        def emit_te(bi):
            di, djs, use_twin = batches[bi]
            w = len(djs)
            c0 = djs[0] if use_twin else djs[0] + LPAD - PAD
            kv = shifted_batch(kps3 if use_twin else kp3, di, c0, w)
            t_t = tpool.tile([128, w, HG, W], BF16, tag="t", name=f"t{bi}")
            q_b = q_sb[:].unsqueeze(1).broadcast_to([128, w, HG, W])
            nc.vector.tensor_mul(t_t[:], q_b, kv)
            e_t = epool.tile([128, w, HG, W], BF16, tag="e", name=f"e{bi}")
            nc.scalar.activation(e_t[:], t_t[:],
                                 mybir.ActivationFunctionType.Exp)
            e_tiles[bi] = e_t
            off_base[bi] = _nxt[0]
            _nxt[0] += w
            den_ps = accref.get("den")
            if den_ps is not None:
                _den_mms(bi, e_t)
                den_done.add(bi)

        def _den_mms(bi, e_t):
            # den += e needs only the exp — emitting it here (not with u)
            # finalizes den ~one batch earlier so the tail reciprocal can
            # overlap PE's trailing num matmuls.
            den_ps = accref["den"]
            w = len(batches[bi][1])
            for j in range(w):
                first = off_base[bi] + j == 0
                last = off_base[bi] + j == n_off - 1
                ej = e_t[:, j].rearrange("p r c -> p (r c)")
                for cc in range(n_mm):
                    sl = slice(cc * mmc, (cc + 1) * mmc)
                    nc.tensor.matmul(den_ps[:, sl], ident_sb[:], ej[:, sl],
                                     start=first, stop=last,
                                     skip_group_check=True)

        den_done = set()

        def emit_u_mm(bi):
            num_ps = accref["num"]
            di, djs, use_twin = batches[bi]
            w = len(djs)
            c0 = djs[0] if use_twin else djs[0] + LPAD - PAD
            vv = shifted_batch(vps3 if use_twin else vp3, di, c0, w)
            e_t = e_tiles.pop(bi)
            if bi not in den_done:
                _den_mms(bi, e_t)
            u_t = upool.tile([128, w, HG, W], BF16, tag="u", name=f"u{bi}")
            nc.vector.tensor_mul(u_t[:], e_t[:], vv)
            for j in range(w):
                first = off_base[bi] + j == 0
                last = off_base[bi] + j == n_off - 1
                uj = u_t[:, j].rearrange("p r c -> p (r c)")
                for cc in range(n_mm):
                    sl = slice(cc * mmc, (cc + 1) * mmc)
                    nc.tensor.matmul(num_ps[:, sl], ident_sb[:], uj[:, sl],
                                     start=first, stop=last,
                                     skip_group_check=True)

        PRO = 3     # prologue batches emitted before the v projection
        for bi in range(PRO):
            emit_te(bi)

        def kv_evac_v(ci, g):
            if g == 0 and ci < 2:
                return nc.vector.tensor_copy
            return nc.scalar.copy
        project_kv(wvT_sb, vp3, "v", kv_evac_v)

        # shifted twins built by the DMA engines (SBUF->SBUF, sync queue),
        # hidden under the first (odd-dj) half of the loop.
        nc.sync.dma_start(kps[:, 0:SLAB - 1], kp[:, 1:SLAB])
        nc.sync.dma_start(vps[:, 0:SLAB - 1], vp[:, 1:SLAB])

        proj_ctx.close()
        warm_ctx.close()
        head.close()

        acc = ctx.enter_context(tc.tile_pool(name="acc", bufs=1, space="PSUM"))
        accref["den"] = acc.tile([128, NHALF], F32, name="den_ps")
        accref["num"] = acc.tile([128, NHALF], F32, name="num_ps")

        # Software-pipeline by LAG batches: the DVE queue is FIFO, so an
        # emission order t_i,u_i would park u_i (whose e_i = exp(t_i) takes
        # ~7us on ACT) at the queue head and stall t_{i+1} behind it.  With
        # t_i,u_{i-LAG}, the exp a u waits on finished ~2 batches ago.
        LAG = 2
        emit_u_mm(0)
        for bi in range(PRO, len(batches)):
            emit_te(bi)
            emit_u_mm(bi - LAG)
        for bi in range(len(batches) - LAG, len(batches)):
            emit_u_mm(bi)

        # --- divide and store ---
        # 1/den on DVE via the fast custom reciprocal (no ACT tables, ACT
        # stays free); per-chunk so recip/mul/DMA overlap the loop drain.
        tail_pool = ctx.enter_context(tc.tile_pool(name="tail", bufs=1))
        den_r = tail_pool.tile([128, NHALF], F32)
        out_sb = tail_pool.tile([128, NHALF], BF16)
        out3 = out_sb.rearrange("p (a b) -> p a b", a=HG)
        for cc in range(4):
            sl = slice(cc * 512, (cc + 1) * 512)
            nc.vector.reciprocal_approx_fast(den_r[:, sl],
                                             accref["den"][:, sl])
            nc.vector.tensor_mul(out_sb[:, sl], accref["num"][:, sl],
                                 den_r[:, sl])
            rsl = slice(cc * 8, (cc + 1) * 8)
            eng = nc.sync if cc % 2 == 0 else nc.scalar
            eng.dma_start(out_d[:, rsl, :], out3[0:64, rsl, :])
            eng.dma_start(out_d[:, HG + cc * 8:HG + (cc + 1) * 8, :],
                          out3[64:128, rsl, :])


_NC_CACHE = None


def _get_nc():
    global _NC_CACHE
    if _NC_CACHE is None:
        _NC_CACHE = build_program()
    return _NC_CACHE


def prepare_in_maps(x, wq, wk, wv):
    x = np.ascontiguousarray(np.asarray(x, dtype=np.float32))
    wqT = np.ascontiguousarray(np.asarray(wq, np.float32).T.astype(_NPBF16))
    wkT = np.ascontiguousarray(np.asarray(wk, np.float32).T.astype(_NPBF16))
    wvT = np.ascontiguousarray(np.asarray(wv, np.float32).T.astype(_NPBF16))
    ident = np.eye(128, dtype=_NPBF16)
    return [
        {"x": x[i].reshape(C, HW), "wqT": wqT, "wkT": wkT, "wvT": wvT,
         "ident": ident}
        for i in range(x.shape[0])
    ]


def run(in_maps, **kw):
    nc = _get_nc()
    return run_bass_kernel_spmd(nc, in_maps, list(range(N_CORES)), **kw)


def kernel(x, wq, wk, wv, rel_w=None, rel_h=None, kernel_size=7, padding=3,
           **_ignored):
    # rel_w/rel_h are constant along the softmax axis, so they cancel.
    assert int(kernel_size) == KS and int(padding) == PAD
    res = run(prepare_in_maps(x, wq, wk, wv))
    out = np.stack([res.results[i]["out"] for i in range(N_CORES)], axis=0)
    return out.astype(np.float32)


if __name__ == "__main__":
    rng = np.random.default_rng(0)
    x = rng.standard_normal((B, C, H, W), dtype=np.float32)
    wq = (rng.standard_normal((O, C)) * 0.1).astype(np.float32)
    wk = (rng.standard_normal((O, C)) * 0.1).astype(np.float32)
    wv = (rng.standard_normal((O, C)) * 0.1).astype(np.float32)
    out = kernel(x, wq, wk, wv)
    print("out", out.shape, out.dtype, float(np.abs(out).max()))
